# revision 15
# baseline (speedup 1.0000x reference)
"""Trainium2 Bass kernel for DeepPoly conv layer (identity-unrolling conv2d).

Problem shapes (hardcoded): C=16, H=W=16, O=32, 3x3 conv, stride 1, pad 1.
Inputs:  x/lower_bound/upper_bound [1,4096] f32, input_shape [4] i32 (unused,
         values known), kernel [32,16,3,3] f32, bias [32] f32.
Outputs: (x_out [1,8192], lb_new [1,8192], ub_new [1,8192],
          weights [4096,8192], bias_vec [8192]).

Math: weights[(c,h,w),(o,h',w')] = kernel[o,c,h-h'+1,w-w'+1] when the tap is
inside the 3x3 window, else 0.  So

    weights[(c,hw),(o,h'w')] = sum_ij kernel[o,c,i,j] * S_ij[hw, h'w']

with S the constant 0/1 shift-pattern table.  On device this is a matmul
with contraction dim 9: lhsT = kmat [9, 512(co)], rhs = S [9, (hw,h'w')].
The kernel values are split hi/lo into two bf16 matmuls accumulating in fp32
PSUM, giving ~1e-5 relative error at full bf16 matmul throughput.

Sharding: each of the 8 cores owns a 32-row slice of hw (the spatial input
position), i.e. rows {c*256 + 32k .. c*256+32k+31 : all c} of weights -> each
core emits 16.8MB (the full output is 134MB).  The tiny bounds propagation
(three 3x3 convs, done as 144-contraction matmuls over host-im2col'd
vectors) is replicated on every core; core 0's copy is returned.
"""

import numpy as np
import ml_dtypes

import concourse.bass as bass  # noqa: F401  (bass types used via tile/bacc)
import concourse.tile as tile
from concourse import bacc, mybir
from concourse.bass_utils import run_bass_kernel_spmd

# ---------------- constants ----------------
C, H, W = 16, 16, 16
O, KH, KW = 32, 3, 3
HW = H * W                # 256
N_IN = C * HW             # 4096
SP = HW                   # 256 output spatial positions (stride 1, pad 1)
M_OUT = O * SP            # 8192
NCORES = 8
HWL = HW // NCORES        # 32 hw rows per core
SFREE = HWL * SP          # 8192 free positions per core
NGROUPS = 4               # co groups of 128 (4 c x 32 o)
NCHUNK = 16               # 512-wide matmul chunks per group
CHUNK = SFREE // NCHUNK   # 512

F32 = mybir.dt.float32
F32R = mybir.dt.float32r
BF16 = mybir.dt.bfloat16
BF = ml_dtypes.bfloat16

_PROG = None


def _build_program():
    nc = bacc.Bacc("TRN2", target_bir_lowering=False, debug=False,
                   num_devices=NCORES)

    s_d = nc.dram_tensor("s_tab", [9, SFREE], F32R, kind="ExternalInput")
    khi_d = nc.dram_tensor("k_hi", [9, NGROUPS * 128], F32R, kind="ExternalInput")
    kb_d = nc.dram_tensor("kb", [72, 6 * O], F32R, kind="ExternalInput")
    xs_d = nc.dram_tensor("xs", [72, 6 * SP], F32R, kind="ExternalInput")
    bias_d = nc.dram_tensor("bias", [O, 1], F32, kind="ExternalInput")

    w_d = nc.dram_tensor("w", [NGROUPS, 128, SFREE], F32, kind="ExternalOutput")
    xo_d = nc.dram_tensor("xo", [O, SP], F32, kind="ExternalOutput")
    lbo_d = nc.dram_tensor("lbo", [O, SP], F32, kind="ExternalOutput")
    ubo_d = nc.dram_tensor("ubo", [O, SP], F32, kind="ExternalOutput")
    bv_d = nc.dram_tensor("bv", [O, SP], F32, kind="ExternalOutput")

    HALFW = SFREE // 2          # 4096 cols per output half-tile
    QW = HALFW // 2             # 2048 cols per PSUM tile (4 banks)
    CPQ = QW // CHUNK           # 4 matmul chunks per PSUM tile

    with tile.TileContext(nc) as tc:
        with (
            tc.tile_pool(name="consts", bufs=1) as consts,
            tc.tile_pool(name="small", bufs=1) as small,
            tc.tile_pool(name="wout", bufs=4) as wpool,
            tc.tile_pool(name="wpsum", bufs=2, space="PSUM") as wpsum,
        ):
            # weights-path inputs first (they gate the matmul pipeline),
            # on the sync HWDGE ring; bounds inputs go on the scalar ring.
            s_sb = consts.tile([9, SFREE], F32R)
            khi_sb = consts.tile([9, NGROUPS * 128], F32R)
            nc.sync.dma_start(out=s_sb[:, :QW], in_=s_d[:, :QW])
            nc.sync.dma_start(out=khi_sb[:, :], in_=khi_d[:, :])
            for piece in range(1, 4):
                nc.sync.dma_start(out=s_sb[:, piece * QW:(piece + 1) * QW],
                                  in_=s_d[:, piece * QW:(piece + 1) * QW])

            kb_sb = consts.tile([72, 6 * O], F32R)
            nc.scalar.dma_start(out=kb_sb[:, :], in_=kb_d[:, :])
            xs_sb = consts.tile([72, 6 * SP], F32R)
            nc.scalar.dma_start(out=xs_sb[:, :], in_=xs_d[:, :])
            bias_sb = consts.tile([O, 1], F32)
            nc.scalar.dma_start(out=bias_sb[:, :], in_=bias_d[:, :])

            # ---- weights generation ----
            # Per quarter (2048 cols = one 4-bank PSUM tile): load khi once,
            # 4 hi-matmuls, load klo, 4 accumulating lo-matmuls, one wide DVE
            # copy.  Two quarters fill a 4096-col half-tile -> one 2MB DMA.
            for g in range(NGROUPS):
                lhs = khi_sb[:, g * 128:(g + 1) * 128]
                for h in range(2):
                    wout = wpool.tile([128, HALFW], F32, tag="wout")
                    for q in range(2):
                        ps = wpsum.tile([128, QW], F32, tag="wps")
                        for t in range(CPQ):
                            tg = (h * 2 + q) * CPQ + t
                            rhs = s_sb[:, tg * CHUNK:(tg + 1) * CHUNK]
                            nc.tensor.matmul(
                                ps[:, t * CHUNK:(t + 1) * CHUNK], lhs, rhs,
                                start=True, stop=True)
                        if q == 0:
                            nc.vector.tensor_copy(
                                wout[:, q * QW:(q + 1) * QW], ps[:, :])
                        else:
                            nc.scalar.copy(
                                wout[:, q * QW:(q + 1) * QW], ps[:, :])
                    nc.sync.dma_start(
                        out=w_d[g][:, h * HALFW:(h + 1) * HALFW],
                        in_=wout[:, :])

            # ---- bounds propagation (tiny, replicated; tail of PE FIFO) ----
            # kb columns: (variant v, half h) block q = v*2+h, 32 cols each;
            # v: 0=full, 1=neg, 2=pos.  xs blocks of 256; v2: 0=x, 1=lb, 2=ub.
            def kbs(v, h):
                q = v * 2 + h
                return kb_sb[:, q * O:(q + 1) * O]

            def xss(v, h):
                q = v * 2 + h
                return xs_sb[:, q * SP:(q + 1) * SP]

            bias3_sb = small.tile([O, 1], F32)
            nc.vector.tensor_scalar_mul(bias3_sb[:, :], bias_sb[:, :], 3.0)

            bv_sb = small.tile([O, SP], F32)
            nc.vector.memset(bv_sb[:, :], 0.0)
            nc.vector.tensor_scalar_add(bv_sb[:, :], bv_sb[:, :], bias_sb[:, 0:1])
            nc.scalar.dma_start(out=bv_d[:, :], in_=bv_sb[:, :])

            for out_d, terms, b_ap in (
                (xo_d, [(0, 0)], bias_sb),
                (lbo_d, [(1, 2), (2, 1)], bias3_sb),
                (ubo_d, [(1, 1), (2, 2)], bias3_sb),
            ):
                ps = wpsum.tile([O, SP], F32, tag="wps")
                mms = [(kv, v2, h) for kv, v2 in terms for h in range(2)]
                for n, (kv, v2, h) in enumerate(mms):
                    nc.tensor.matmul(ps[:, :], kbs(kv, h), xss(v2, h),
                                     start=(n == 0), stop=(n == len(mms) - 1))
                o_sb = small.tile([O, SP], F32, tag="bout")
                nc.vector.tensor_scalar_add(o_sb[:, :], ps[:, :], b_ap[:, 0:1])
                nc.scalar.dma_start(out=out_d[:, :], in_=o_sb[:, :])

    nc.compile()
    return nc


def build_program():
    global _PROG
    if _PROG is None:
        _PROG = _build_program()
    return _PROG


# ---------------- host-side data prep ----------------

def _make_s_table(core):
    """S_k[ij, hw_l, h'w'] in {0,1}: 1 iff basis pixel hw=32*core+hw_l
    contributes via tap (i,j) to output position (h',w')."""
    s = np.zeros((9, HWL, SP), dtype=np.float32)
    for ij in range(9):
        i, j = divmod(ij, 3)
        for hw_l in range(HWL):
            hw = HWL * core + hw_l
            h, w = divmod(hw, W)
            hp, wp = h - i + 1, w - j + 1
            if 0 <= hp < H and 0 <= wp < W:
                s[ij, hw_l, hp * W + wp] = 1.0
    return np.ascontiguousarray(s.reshape(9, SFREE))


def _im2col(img):
    """img [C,H,W] f32 -> [2, 72, 256]: xs[half, c_l*9+ij, h'w'] =
    img[half*8+c_l, h'+i-1, w'+j-1] (zero padded)."""
    pad = np.zeros((C, H + 2, W + 2), np.float32)
    pad[:, 1:H + 1, 1:W + 1] = img
    out = np.zeros((2, 72, SP), np.float32)
    for c in range(C):
        for i in range(KH):
            for j in range(KW):
                patch = pad[c, i:i + H, j:j + W]
                out[c // 8, (c % 8) * 9 + i * 3 + j, :] = patch.reshape(SP)
    return out


def build_in_maps(inputs):
    x = np.asarray(inputs["x"], np.float32).reshape(C, H, W)
    lb = np.asarray(inputs["lower_bound"], np.float32).reshape(C, H, W)
    ub = np.asarray(inputs["upper_bound"], np.float32).reshape(C, H, W)
    kern = np.asarray(inputs["kernel"], np.float32)
    bias = np.asarray(inputs["bias"], np.float32)

    # weights-gen lhsT: kmat[ij, c*32+o] = kern[o,c,i,j]
    kmat = np.ascontiguousarray(kern.transpose(2, 3, 1, 0).reshape(9, C * O))

    # bounds lhsT: kb[(c_l,i,j), (v,half,o)]
    kfull = kern
    kneg = np.where(kern < 0, kern, 0.0).astype(np.float32)
    kpos = np.where(kern >= 0, kern, 0.0).astype(np.float32)
    kb = np.zeros((72, 6, O), np.float32)
    for v, kv in enumerate([kfull, kneg, kpos]):
        for half in range(2):
            kb[:, v * 2 + half, :] = (
                kv[:, half * 8:(half + 1) * 8, :, :]
                .transpose(1, 2, 3, 0).reshape(72, O)
            )
    kb = np.ascontiguousarray(kb.reshape(72, 6 * O))

    # bounds rhs: xs[(c_l,i,j), (v2,half,s')], v2: 0=x 1=lb 2=ub
    xs = np.zeros((72, 6, SP), np.float32)
    for v2, img in enumerate([x, lb, ub]):
        cols = _im2col(img)
        xs[:, v2 * 2 + 0, :] = cols[0]
        xs[:, v2 * 2 + 1, :] = cols[1]
    xs = np.ascontiguousarray(xs.reshape(72, 6 * SP))

    bias2 = np.ascontiguousarray(bias.reshape(O, 1))

    in_maps = []
    for k in range(NCORES):
        in_maps.append({
            "s_tab": _make_s_table(k),
            "k_hi": kmat,
            "kb": kb,
            "xs": xs,
            "bias": bias2,
        })
    return in_maps


def assemble(results):
    # weights: per core arr [g, c_l, o, hw_l, s'] -> [c, hw_l, (o s')]
    w3 = np.empty((C, HW, M_OUT), np.float32)
    for k in range(NCORES):
        arr = results[k]["w"].reshape(NGROUPS, 4, O, HWL, SP)
        part = arr.transpose(0, 1, 3, 2, 4).reshape(C, HWL, M_OUT)
        w3[:, HWL * k:HWL * (k + 1), :] = part
    weights = w3.reshape(N_IN, M_OUT)

    r0 = results[0]
    x_out = r0["xo"].reshape(1, M_OUT).astype(np.float32)
    lb_new = r0["lbo"].reshape(1, M_OUT).astype(np.float32)
    ub_new = r0["ubo"].reshape(1, M_OUT).astype(np.float32)
    bias_vec = r0["bv"].reshape(M_OUT).astype(np.float32)
    return (x_out, lb_new, ub_new, weights, bias_vec)


def kernel(**inputs):
    nc = build_program()
    in_maps = build_in_maps(inputs)
    res = run_bass_kernel_spmd(nc, in_maps, core_ids=list(range(NCORES)))
    return assemble(res.results)


# revision 21
# speedup vs baseline: 1.1294x; 1.1294x over previous
"""Trainium2 Bass kernel for DeepPoly conv layer (identity-unrolling conv2d).

Problem shapes (hardcoded): C=16, H=W=16, O=32, 3x3 conv, stride 1, pad 1.
Inputs:  x/lower_bound/upper_bound [1,4096] f32, input_shape [4] i32 (unused,
         values known), kernel [32,16,3,3] f32, bias [32] f32.
Outputs: (x_out [1,8192], lb_new [1,8192], ub_new [1,8192],
          weights [4096,8192], bias_vec [8192]).

Math: weights[(c,h,w),(o,h',w')] = kernel[o,c,h-h'+1,w-w'+1] when the tap is
inside the 3x3 window, else 0.  So

    weights[(c,hw),(o,h'w')] = sum_ij kernel[o,c,i,j] * S_ij[hw, h'w']

with S the constant 0/1 shift-pattern table.  On device this is a matmul
with contraction dim 9: lhsT = kmat [9, 512(co)], rhs = S [9, (hw,h'w')].
The kernel values are split hi/lo into two bf16 matmuls accumulating in fp32
PSUM, giving ~1e-5 relative error at full bf16 matmul throughput.

Sharding: each of the 8 cores owns a 32-row slice of hw (the spatial input
position), i.e. rows {c*256 + 32k .. c*256+32k+31 : all c} of weights -> each
core emits 16.8MB (the full output is 134MB).  The tiny bounds propagation
(three 3x3 convs, done as 144-contraction matmuls over host-im2col'd
vectors) is replicated on every core; core 0's copy is returned.
"""

import numpy as np
import ml_dtypes

import concourse.bass as bass  # noqa: F401  (bass types used via tile/bacc)
import concourse.tile as tile
from concourse import bacc, mybir
from concourse.bass_utils import run_bass_kernel_spmd

# ---------------- constants ----------------
C, H, W = 16, 16, 16
O, KH, KW = 32, 3, 3
HW = H * W                # 256
N_IN = C * HW             # 4096
SP = HW                   # 256 output spatial positions (stride 1, pad 1)
M_OUT = O * SP            # 8192
NCORES = 8
HWL = HW // NCORES        # 32 hw rows per core
SFREE = HWL * SP          # 8192 free positions per core
NGROUPS = 4               # co groups of 128 (4 c x 32 o)
NCHUNK = 16               # 512-wide matmul chunks per group
CHUNK = SFREE // NCHUNK   # 512

F32 = mybir.dt.float32
F32R = mybir.dt.float32r
BF16 = mybir.dt.bfloat16
BF = ml_dtypes.bfloat16

_PROG = None


def _build_program():
    nc = bacc.Bacc("TRN2", target_bir_lowering=False, debug=False,
                   num_devices=NCORES)

    # ks packs the weights-matmul lhsT (cols 0..511) and the shift table S
    # (cols 512..) so one DMA unblocks the first matmul.
    ks_d = nc.dram_tensor("ks", [9, 512 + SFREE], F32R, kind="ExternalInput")
    kb_d = nc.dram_tensor("kb", [72, 6 * O], F32R, kind="ExternalInput")
    xs_d = nc.dram_tensor("xs", [72, 6 * SP], F32R, kind="ExternalInput")
    bias_d = nc.dram_tensor("bias", [O, 1], F32, kind="ExternalInput")

    w_d = nc.dram_tensor("w", [NGROUPS, 128, SFREE], F32, kind="ExternalOutput")
    xo_d = nc.dram_tensor("xo", [O, SP], F32, kind="ExternalOutput")
    lbo_d = nc.dram_tensor("lbo", [O, SP], F32, kind="ExternalOutput")
    ubo_d = nc.dram_tensor("ubo", [O, SP], F32, kind="ExternalOutput")
    bv_d = nc.dram_tensor("bv", [O, SP], F32, kind="ExternalOutput")

    HALFW = SFREE // 2          # 4096 cols per output half-tile
    QW = HALFW // 2             # 2048 cols per PSUM tile (4 banks)
    CPQ = QW // CHUNK           # 4 matmul chunks per PSUM tile

    with tile.TileContext(nc) as tc:
        with (
            tc.tile_pool(name="consts", bufs=1) as consts,
            tc.tile_pool(name="small", bufs=1) as small,
            tc.tile_pool(name="wout", bufs=4) as wpool,
            tc.tile_pool(name="wpsum", bufs=4, space="PSUM") as wpsum,
        ):
            # weights-path inputs first (they gate the matmul pipeline),
            # on the sync HWDGE ring; bounds inputs go on the scalar ring.
            ks_sb = consts.tile([9, 512 + SFREE], F32R)
            nc.sync.dma_start(out=ks_sb[:, :512 + QW], in_=ks_d[:, :512 + QW])
            for piece in range(1, 4):
                sl = slice(512 + piece * QW, 512 + (piece + 1) * QW)
                nc.sync.dma_start(out=ks_sb[:, sl], in_=ks_d[:, sl])

            kb_sb = consts.tile([72, 6 * O], F32R)
            nc.gpsimd.dma_start(out=kb_sb[:, :], in_=kb_d[:, :])
            xs_sb = consts.tile([72, 6 * SP], F32R)
            nc.gpsimd.dma_start(out=xs_sb[:, :], in_=xs_d[:, :])
            bias_sb = consts.tile([O, 1], F32)
            nc.gpsimd.dma_start(out=bias_sb[:, :], in_=bias_d[:, :])

            # ---- weights generation ----
            # Per quarter (2048 cols = one 4-bank PSUM tile): load khi once,
            # 4 hi-matmuls, load klo, 4 accumulating lo-matmuls, one wide DVE
            # copy.  Two quarters fill a 4096-col half-tile -> one 2MB DMA.
            QTR = 1024               # psum tile width: 2 chunks, 2 banks
            for g in range(NGROUPS):
                lhs = ks_sb[:, g * 128:(g + 1) * 128]
                for h in range(2):
                    wout = wpool.tile([128, HALFW], F32, tag="wout")
                    for q in range(4):
                        ps = wpsum.tile([128, QTR], F32, tag="wps")
                        for t in range(2):
                            tg = (h * 4 + q) * 2 + t
                            rhs = ks_sb[:, 512 + tg * CHUNK:512 + (tg + 1) * CHUNK]
                            nc.tensor.matmul(
                                ps[:, t * CHUNK:(t + 1) * CHUNK], lhs, rhs,
                                start=True, stop=True)
                        dst = wout[:, q * QTR:(q + 1) * QTR]
                        if q % 2 == 0:
                            nc.vector.tensor_copy(dst, ps[:, :])
                        else:
                            nc.scalar.copy(dst, ps[:, :])
                    if g == 0 and h == 0:
                        # first half as 2x1MB so the output stream starts early
                        nc.sync.dma_start(out=w_d[0][:, :HALFW // 2],
                                          in_=wout[:, :HALFW // 2])
                        nc.sync.dma_start(out=w_d[0][:, HALFW // 2:HALFW],
                                          in_=wout[:, HALFW // 2:])
                    else:
                        nc.sync.dma_start(
                            out=w_d[g][:, h * HALFW:(h + 1) * HALFW],
                            in_=wout[:, :])

            # ---- bounds propagation (tiny, replicated; tail of PE FIFO) ----
            # kb columns: (variant v, half h) block q = v*2+h, 32 cols each;
            # v: 0=full, 1=neg, 2=pos.  xs blocks of 256; v2: 0=x, 1=lb, 2=ub.
            def kbs(v, h):
                q = v * 2 + h
                return kb_sb[:, q * O:(q + 1) * O]

            def xss(v, h):
                q = v * 2 + h
                return xs_sb[:, q * SP:(q + 1) * SP]

            bias3_sb = small.tile([O, 1], F32)
            nc.vector.tensor_scalar_mul(bias3_sb[:, :], bias_sb[:, :], 3.0)

            bv_sb = small.tile([O, SP], F32)
            nc.vector.memset(bv_sb[:, :], 0.0)
            nc.vector.tensor_scalar_add(bv_sb[:, :], bv_sb[:, :], bias_sb[:, 0:1])
            nc.scalar.dma_start(out=bv_d[:, :], in_=bv_sb[:, :])

            for out_d, terms, b_ap in (
                (xo_d, [(0, 0)], bias_sb),
                (lbo_d, [(1, 2), (2, 1)], bias3_sb),
                (ubo_d, [(1, 1), (2, 2)], bias3_sb),
            ):
                ps = wpsum.tile([O, SP], F32, tag="wps")
                mms = [(kv, v2, h) for kv, v2 in terms for h in range(2)]
                for n, (kv, v2, h) in enumerate(mms):
                    nc.tensor.matmul(ps[:, :], kbs(kv, h), xss(v2, h),
                                     start=(n == 0), stop=(n == len(mms) - 1))
                o_sb = small.tile([O, SP], F32, tag="bout")
                nc.vector.tensor_scalar_add(o_sb[:, :], ps[:, :], b_ap[:, 0:1])
                nc.scalar.dma_start(out=out_d[:, :], in_=o_sb[:, :])

    nc.compile()
    return nc


def build_program():
    global _PROG
    if _PROG is None:
        _PROG = _build_program()
    return _PROG


# ---------------- host-side data prep ----------------

def _make_s_table(core):
    """S_k[ij, hw_l, h'w'] in {0,1}: 1 iff basis pixel hw=32*core+hw_l
    contributes via tap (i,j) to output position (h',w')."""
    s = np.zeros((9, HWL, SP), dtype=np.float32)
    for ij in range(9):
        i, j = divmod(ij, 3)
        for hw_l in range(HWL):
            hw = HWL * core + hw_l
            h, w = divmod(hw, W)
            hp, wp = h - i + 1, w - j + 1
            if 0 <= hp < H and 0 <= wp < W:
                s[ij, hw_l, hp * W + wp] = 1.0
    return np.ascontiguousarray(s.reshape(9, SFREE))


def _im2col(img):
    """img [C,H,W] f32 -> [2, 72, 256]: xs[half, c_l*9+ij, h'w'] =
    img[half*8+c_l, h'+i-1, w'+j-1] (zero padded)."""
    pad = np.zeros((C, H + 2, W + 2), np.float32)
    pad[:, 1:H + 1, 1:W + 1] = img
    out = np.zeros((2, 72, SP), np.float32)
    for c in range(C):
        for i in range(KH):
            for j in range(KW):
                patch = pad[c, i:i + H, j:j + W]
                out[c // 8, (c % 8) * 9 + i * 3 + j, :] = patch.reshape(SP)
    return out


def build_in_maps(inputs):
    x = np.asarray(inputs["x"], np.float32).reshape(C, H, W)
    lb = np.asarray(inputs["lower_bound"], np.float32).reshape(C, H, W)
    ub = np.asarray(inputs["upper_bound"], np.float32).reshape(C, H, W)
    kern = np.asarray(inputs["kernel"], np.float32)
    bias = np.asarray(inputs["bias"], np.float32)

    # weights-gen lhsT: kmat[ij, c*32+o] = kern[o,c,i,j]
    kmat = np.ascontiguousarray(kern.transpose(2, 3, 1, 0).reshape(9, C * O))

    # bounds lhsT: kb[(c_l,i,j), (v,half,o)]
    kfull = kern
    kneg = np.where(kern < 0, kern, 0.0).astype(np.float32)
    kpos = np.where(kern >= 0, kern, 0.0).astype(np.float32)
    kb = np.zeros((72, 6, O), np.float32)
    for v, kv in enumerate([kfull, kneg, kpos]):
        for half in range(2):
            kb[:, v * 2 + half, :] = (
                kv[:, half * 8:(half + 1) * 8, :, :]
                .transpose(1, 2, 3, 0).reshape(72, O)
            )
    kb = np.ascontiguousarray(kb.reshape(72, 6 * O))

    # bounds rhs: xs[(c_l,i,j), (v2,half,s')], v2: 0=x 1=lb 2=ub
    xs = np.zeros((72, 6, SP), np.float32)
    for v2, img in enumerate([x, lb, ub]):
        cols = _im2col(img)
        xs[:, v2 * 2 + 0, :] = cols[0]
        xs[:, v2 * 2 + 1, :] = cols[1]
    xs = np.ascontiguousarray(xs.reshape(72, 6 * SP))

    bias2 = np.ascontiguousarray(bias.reshape(O, 1))

    in_maps = []
    for k in range(NCORES):
        in_maps.append({
            "ks": np.ascontiguousarray(
                np.concatenate([kmat, _make_s_table(k)], axis=1)),
            "kb": kb,
            "xs": xs,
            "bias": bias2,
        })
    return in_maps


def assemble(results):
    # weights: per core arr [g, c_l, o, hw_l, s'] -> [c, hw_l, (o s')]
    w3 = np.empty((C, HW, M_OUT), np.float32)
    for k in range(NCORES):
        arr = results[k]["w"].reshape(NGROUPS, 4, O, HWL, SP)
        part = arr.transpose(0, 1, 3, 2, 4).reshape(C, HWL, M_OUT)
        w3[:, HWL * k:HWL * (k + 1), :] = part
    weights = w3.reshape(N_IN, M_OUT)

    r0 = results[0]
    x_out = r0["xo"].reshape(1, M_OUT).astype(np.float32)
    lb_new = r0["lbo"].reshape(1, M_OUT).astype(np.float32)
    ub_new = r0["ubo"].reshape(1, M_OUT).astype(np.float32)
    bias_vec = r0["bv"].reshape(M_OUT).astype(np.float32)
    return (x_out, lb_new, ub_new, weights, bias_vec)


def kernel(**inputs):
    nc = build_program()
    in_maps = build_in_maps(inputs)
    res = run_bass_kernel_spmd(nc, in_maps, core_ids=list(range(NCORES)))
    return assemble(res.results)
